# revision 33
# baseline (speedup 1.0000x reference)
"""MoE layer (8 experts, top-2) on 8 Trainium2 NeuronCores, expert-parallel.

Strategy
--------
Host (dispatch): compute router logits/top-k on host, gather each expert's
tokens into a padded capacity buffer (C = max expert load, 4-aligned),
pre-pack activations/weights into the exact SBUF tile layout
(partition-major) in fp16.
Device (one expert per core, SPMD): Y_e = w_down[e] @ (silu(w_gate[e] @ x_e)
* (w_up[e] @ x_e)) over the expert's C gathered tokens; all matmuls fp16
inputs with fp32 PSUM accumulation (fp16 runs at full PE rate like bf16 but
with 8x finer mantissa). Token columns are processed in 512-wide blocks;
the remainder is merged into the last block's weight pass and the merged
pair rebalanced so no block drops below the ~69-col instruction-issue
floor of the PE (measured: the tensor engine issues matmuls no faster
than ~29ns apart, and a 512-col fp16 matmul takes 216ns = 2.37GHz).
Host (combine): scatter-add per-token routing-weighted outputs.

Trace-derived tuning notes (this exact workload, TRN2):
- Only sync (qSPDynamicHW) and scalar (qActDynamicHW) issue HW-DGE DMAs;
  gpsimd DMA is software-DGE and far too slow for streaming.
- All heavy weight streams must ride sync: scalar runs the ACTIVATE
  (silu) instructions, and DMA issues blocked on semaphore-slot reuse
  would delay them, stalling PSUM recycling and the PE.
- Every weight pass needs per-m-iter compute >= the sync ring's ~5-6us
  per-m weight delivery, hence all non-tail blocks stay 512 wide.
"""

import os
import numpy as np
from contextlib import ExitStack

H = 2048
I = 5632
E = 8
P = 128
NB = 512  # token block (matmul free dim / PSUM bank)

KH = H // P   # 16  k-tiles over H
MI = I // P   # 44  m-tiles over I

DT = np.float16  # fp16: PE full rate like bf16, 8x finer mantissa


def _superblocks(C):
    """Column groups; a trailing remainder (<NB) is merged into the last
    full block so both share one pass over the weights.

    Matmuls below ~69 cols are bound by the 29ns instruction-issue floor
    (29ns buys 69 cols at 2.37GHz), so a skinny tail wastes PE time.
    Non-tail passes must stay 512 wide: a narrower pass consumes weights
    faster than the single sync HW-DGE ring delivers (~5-6us per m-iter),
    and the weight stream cannot ride the scalar ring without burying the
    ACTIVATE instructions behind blocking DMA waits.  So rebalance only
    inside the merged tail pass: [512, t<128] -> [384+t, 128]."""
    blocks = []
    t = 0
    while t < C:
        blocks.append((t, min(NB, C - t)))
        t += NB
    sbs = [[b] for b in blocks]
    if len(sbs) >= 2 and sbs[-1][0][1] < NB:
        tail = sbs.pop()[0]
        sbs[-1].append(tail)
        (t0, w0), (t1, w1) = sbs[-1]
        if w1 < 128:
            w0n = w0 + w1 - 128
            sbs[-1] = [(t0, w0n), (t0 + w0n, 128)]
    return sbs


def build_program(C, h=H, i_dim=I, sim_safe_act=False):
    """Build the SPMD bass program for one expert over C tokens.

    DRAM I/O layouts (all partition-major, pre-packed on host):
      x  [P, KH, C]        fp16   x[p, k, t]  = token t, hidden 128k+p
      wg [MI, P, KH*P]     fp16   wg[m, p, kf] (kf = k*128+f): w_gate.T tiles
      wu [MI, P, KH*P]     fp16   same for w_up
      wd [KH, P, MI*P]     fp16   w_down.T tiles
      y  [P, KH, C]        bf16   y[p, m2, t] = output hidden 128*m2+p
           (bf16 keeps y's ~0.2% quantization noise far under the 2e-2
            gate and halves the final drain + scalar-ring write traffic)
    """
    from concourse import bacc, tile, mybir

    kh = h // P
    mi = i_dim // P
    bf = mybir.dt.float16
    bf16 = mybir.dt.bfloat16
    f32 = mybir.dt.float32
    Silu = mybir.ActivationFunctionType.Silu

    nc = bacc.Bacc(None)
    X = nc.declare_dram_parameter("x", [P, kh, C], bf, isOutput=False)
    WG = nc.declare_dram_parameter("wg", [mi, P, kh * P], bf, isOutput=False)
    WU = nc.declare_dram_parameter("wu", [mi, P, kh * P], bf, isOutput=False)
    WD = nc.declare_dram_parameter("wd", [kh, P, mi * P], bf, isOutput=False)
    Y = nc.declare_dram_parameter("y", [P, kh, C], bf16, isOutput=True)

    with ExitStack() as ctx:
        tc = ctx.enter_context(tile.TileContext(nc))
        xpool = ctx.enter_context(tc.tile_pool(name="xpool", bufs=2))
        wpool = ctx.enter_context(tc.tile_pool(name="wpool", bufs=6))
        dpool = ctx.enter_context(tc.tile_pool(name="dpool", bufs=4))
        hpool = ctx.enter_context(tc.tile_pool(name="hpool", bufs=1))
        apool = ctx.enter_context(tc.tile_pool(name="apool", bufs=3))
        ypool = ctx.enter_context(tc.tile_pool(name="ypool", bufs=3))
        pg_pool = ctx.enter_context(tc.tile_pool(name="pg", bufs=3, space="PSUM"))
        pu_pool = ctx.enter_context(tc.tile_pool(name="pu", bufs=3, space="PSUM"))
        py_pool = ctx.enter_context(tc.tile_pool(name="py", bufs=2, space="PSUM"))

        first_sb = True
        for sb in _superblocks(C):
            # Only sync (qSPDynamicHW) and scalar (qActDynamicHW) are
            # hardware DGE rings; gpsimd DMA is software-DGE and slow.
            q = kh * P // 4
            pre_wg = pre_wu = None
            x_ts = []
            if first_sb:
                # ---- first superblock: interleave the m=0 weight chunks
                # with the x chunks across both HW rings so the first pg
                # chain starts at the ~12us DMA-latency floor instead of
                # queueing all 16 x chunks ahead of the weights (~19us).
                (t0, tn) = sb[0]
                x_t = xpool.tile([P, kh, tn], bf, tag="x_t0", name="x_t0")
                x_ts.append(x_t)
                pre_wg = wpool.tile([P, kh * P], bf, tag="wg_t")
                pre_wu = wpool.tile([P, kh * P], bf, tag="wu_t")
                for j in range(4):
                    nc.sync.dma_start(pre_wg[:, j * q : (j + 1) * q], WG[0, :, j * q : (j + 1) * q])
                    if j == 0:
                        # x[k=0] first on scalar: the first matmul gates on
                        # wg-c0 (sync pos 1) AND x0 — land both in parallel
                        nc.scalar.dma_start(x_t[:, 0, :tn], X[:, 0, t0 : t0 + tn])
                        nc.scalar.dma_start(pre_wu[:, 0:q], WU[0, :, 0:q])
                        for k in (1, 2, 3):
                            eng = nc.sync if k % 2 == 1 else nc.scalar
                            eng.dma_start(x_t[:, k, :tn], X[:, k, t0 : t0 + tn])
                    else:
                        nc.scalar.dma_start(pre_wu[:, j * q : (j + 1) * q], WU[0, :, j * q : (j + 1) * q])
                        for k in range(4 * j, 4 * j + 4):
                            eng = nc.sync if k % 2 == 0 else nc.scalar
                            eng.dma_start(x_t[:, k, :tn], X[:, k, t0 : t0 + tn])
            else:
                # ---- load X for each column group: kh tiles [P, tn]
                for g, (t0, tn) in enumerate(sb):
                    x_t = xpool.tile([P, kh, tn], bf, tag=f"x_t{g}", name=f"x_t{g}")
                    for k in range(kh):
                        eng = nc.scalar if k % 2 == 0 else nc.sync
                        eng.dma_start(x_t[:, k, :tn], X[:, k, t0 : t0 + tn])
                    x_ts.append(x_t)
            first_sb = False

            # ---- mm1/mm2 + silu*mul -> h (one weight pass for all groups)
            h_ts = [
                hpool.tile([P, mi, sb[g][1]], bf, tag=f"h{g}", name=f"h_t{g}")
                for g in range(len(sb))
            ]
            for m in range(mi):
                if m == 0 and pre_wg is not None:
                    wg_t, wu_t = pre_wg, pre_wu
                else:
                    # all weights on sync: it is the one HW-DGE ring with no
                    # compute duties, so its blocking DMA waits hurt nothing
                    wg_t = wpool.tile([P, kh * P], bf, tag="wg_t")
                    for j in range(4):
                        nc.sync.dma_start(wg_t[:, j * q : (j + 1) * q], WG[m, :, j * q : (j + 1) * q])
                    wu_t = wpool.tile([P, kh * P], bf, tag="wu_t")
                    for j in range(4):
                        nc.sync.dma_start(wu_t[:, j * q : (j + 1) * q], WU[m, :, j * q : (j + 1) * q])

                pgs, pus = [], []
                for g, (t0, tn) in enumerate(sb):
                    pg = pg_pool.tile([P, NB], f32, tag="pg")
                    pgs.append(pg)
                    for k in range(kh):
                        nc.tensor.matmul(
                            pg[:, :tn],
                            wg_t[:, k * P : (k + 1) * P],
                            x_ts[g][:, k, :tn],
                            start=(k == 0),
                            stop=(k == kh - 1),
                        )
                for g, (t0, tn) in enumerate(sb):
                    pu = pu_pool.tile([P, NB], f32, tag="pu")
                    pus.append(pu)
                    for k in range(kh):
                        nc.tensor.matmul(
                            pu[:, :tn],
                            wu_t[:, k * P : (k + 1) * P],
                            x_ts[g][:, k, :tn],
                            start=(k == 0),
                            stop=(k == kh - 1),
                        )
                for g, (t0, tn) in enumerate(sb):
                    pg, pu = pgs[g], pus[g]
                    g_act = apool.tile([P, NB], f32, tag="g_act")
                    if sim_safe_act:
                        # silu(g) = g * sigmoid(g); CoreSim lacks the Silu LUT
                        nc.scalar.activation(
                            g_act[:, :tn],
                            pg[:, :tn],
                            mybir.ActivationFunctionType.Sigmoid,
                        )
                        nc.vector.tensor_mul(g_act[:, :tn], g_act[:, :tn], pg[:, :tn])
                    else:
                        nc.scalar.activation(g_act[:, :tn], pg[:, :tn], Silu)
                    nc.vector.tensor_mul(h_ts[g][:, m, :tn], g_act[:, :tn], pu[:, :tn])

            # ---- mm3 -> y (one weight pass for all groups)
            for m2 in range(kh):
                dhalf = mi * P // 2
                wd_t = dpool.tile([P, mi * P], bf, tag="wd_t")
                nc.sync.dma_start(wd_t[:, :dhalf], WD[m2, :, :dhalf])
                nc.sync.dma_start(wd_t[:, dhalf:], WD[m2, :, dhalf:])
                # tail group first so its drain hides behind the main
                # stream — except on the very last m2, where main-first
                # leaves only the small tail tile's drain exposed at the end
                g_order = list(enumerate(sb))
                if m2 < kh - 1:
                    g_order = list(reversed(g_order))
                for g, (t0, tn) in g_order:
                    py = py_pool.tile([P, NB], f32, tag="py")
                    for k2 in range(mi):
                        nc.tensor.matmul(
                            py[:, :tn],
                            wd_t[:, k2 * P : (k2 + 1) * P],
                            h_ts[g][:, k2, :tn],
                            start=(k2 == 0),
                            stop=(k2 == mi - 1),
                        )
                    y_sb = ypool.tile([P, NB], bf16, tag="y_sb")
                    nc.vector.tensor_copy(y_sb[:, :tn], py[:, :tn])
                    nc.scalar.dma_start(Y[:, m2, t0 : t0 + tn], y_sb[:, :tn])

    nc.compile()
    return nc


def _route(xf, gate_w, top_k):
    """Host router: returns per-expert (token_indices, weights)."""
    logits = xf @ gate_w.T.astype(np.float32)  # [T, E]
    m = logits.max(-1, keepdims=True)
    p = np.exp(logits - m)
    p /= p.sum(-1, keepdims=True)
    k = int(top_k)
    if k >= E:
        top_i = np.tile(np.arange(E), (xf.shape[0], 1))
    else:
        top_i = np.argpartition(-p, k, axis=-1)[:, :k]
    top_w = np.take_along_axis(p, top_i, axis=-1)
    top_w = top_w / top_w.sum(-1, keepdims=True)
    idxs, wts = [], []
    for e in range(E):
        sel = top_i == e  # [T, k]
        tok = np.nonzero(sel.any(-1))[0]
        w = (top_w * sel).sum(-1)[tok].astype(np.float32)
        idxs.append(tok)
        wts.append(w)
    return idxs, wts


def _pack_w1(w):  # [I, H] -> [MI, P, KH*P]; lhsT tile (m,k)[p,f] = w[128m+f, 128k+p]
    return np.ascontiguousarray(
        w.reshape(MI, P, KH, P).transpose(0, 3, 2, 1).reshape(MI, P, KH * P)
    )


def _pack_w3(w):  # [H, I] -> [KH, P, MI*P]; lhsT tile (m2,k2)[p,f] = w[128m2+f, 128k2+p]
    return np.ascontiguousarray(
        w.reshape(KH, P, MI, P).transpose(0, 3, 2, 1).reshape(KH, P, MI * P)
    )


def kernel(x, gate_w, w_gate, w_up, w_down, top_k):
    from concourse.bass_utils import run_bass_kernel_spmd

    x = np.asarray(x, dtype=np.float32)
    gate_w = np.asarray(gate_w, dtype=np.float32)
    w_gate = np.asarray(w_gate, dtype=np.float32)
    w_up = np.asarray(w_up, dtype=np.float32)
    w_down = np.asarray(w_down, dtype=np.float32)
    shape = x.shape
    xf = x.reshape(-1, shape[-1])
    T = xf.shape[0]

    idxs, wts = _route(xf, gate_w, top_k)
    C = max(max(len(ix) for ix in idxs), NB)
    C = ((C + 3) // 4) * 4  # pad only to 4 (8B DMA lines) — C is the roofline

    nc = build_program(C)

    xf_bf = xf.astype(DT)
    in_maps = []
    for e in range(E):
        tok = idxs[e]
        xg = np.zeros((C, H), dtype=DT)
        xg[: len(tok)] = xf_bf[tok]
        # [C, H] -> x[p, k, t] = xg[t, 128k+p]
        xp = np.ascontiguousarray(xg.reshape(C, KH, P).transpose(2, 1, 0))
        in_maps.append(
            {
                "x": xp,
                "wg": _pack_w1(w_gate[e].astype(DT)),
                "wu": _pack_w1(w_up[e].astype(DT)),
                "wd": _pack_w3(w_down[e].astype(DT)),
            }
        )

    trace = bool(os.environ.get("BASS_TRACE"))
    if trace:
        try:
            import antenv.axon_hooks  # noqa: F401  (trace path needs it under axon)
        except ImportError:
            trace = False
            os.environ["BASS_NEVER_TRACE"] = "1"
    res = run_bass_kernel_spmd(nc, in_maps, list(range(E)), trace=trace)
    globals()["LAST_RESULT"] = res

    out = np.zeros((T, H), dtype=np.float32)
    for e in range(E):
        tok = idxs[e]
        y = res.results[e]["y"].astype(np.float32)  # [P, KH, C] bf16 on device
        yt = y.transpose(2, 1, 0).reshape(C, H)[: len(tok)]
        out[tok] += yt * wts[e][:, None]
    return out.reshape(shape)



# revision 34
# speedup vs baseline: 1.0015x; 1.0015x over previous
"""MoE layer (8 experts, top-2) on 8 Trainium2 NeuronCores, expert-parallel.

Strategy
--------
Host (dispatch): compute router logits/top-k on host, gather each expert's
tokens into a padded capacity buffer (C = max expert load, 4-aligned),
pre-pack activations/weights into the exact SBUF tile layout
(partition-major) in fp16.
Device (one expert per core, SPMD): Y_e = w_down[e] @ (silu(w_gate[e] @ x_e)
* (w_up[e] @ x_e)) over the expert's C gathered tokens; all matmuls fp16
inputs with fp32 PSUM accumulation (fp16 runs at full PE rate like bf16 but
with 8x finer mantissa). Token columns are processed in 512-wide blocks;
the remainder is merged into the last block's weight pass and the merged
pair rebalanced so no block drops below the ~69-col instruction-issue
floor of the PE (measured: the tensor engine issues matmuls no faster
than ~29ns apart, and a 512-col fp16 matmul takes 216ns = 2.37GHz).
Host (combine): scatter-add per-token routing-weighted outputs.

Trace-derived tuning notes (this exact workload, TRN2):
- Only sync (qSPDynamicHW) and scalar (qActDynamicHW) issue HW-DGE DMAs;
  gpsimd DMA is software-DGE and far too slow for streaming.
- All heavy weight streams must ride sync: scalar runs the ACTIVATE
  (silu) instructions, and DMA issues blocked on semaphore-slot reuse
  would delay them, stalling PSUM recycling and the PE.
- Every weight pass needs per-m-iter compute >= the sync ring's ~5-6us
  per-m weight delivery, hence all non-tail blocks stay 512 wide.
"""

import os
import numpy as np
from contextlib import ExitStack

H = 2048
I = 5632
E = 8
P = 128
NB = 512  # token block (matmul free dim / PSUM bank)

KH = H // P   # 16  k-tiles over H
MI = I // P   # 44  m-tiles over I

DT = np.float16  # fp16: PE full rate like bf16, 8x finer mantissa


def _superblocks(C):
    """Column groups; a trailing remainder (<NB) is merged into the last
    full block so both share one pass over the weights.

    Matmuls below ~69 cols are bound by the 29ns instruction-issue floor
    (29ns buys 69 cols at 2.37GHz), so a skinny tail wastes PE time.
    Non-tail passes must stay 512 wide: a narrower pass consumes weights
    faster than the single sync HW-DGE ring delivers (~5-6us per m-iter),
    and the weight stream cannot ride the scalar ring without burying the
    ACTIVATE instructions behind blocking DMA waits.  So rebalance only
    inside the merged tail pass: [512, t<128] -> [384+t, 128]."""
    blocks = []
    t = 0
    while t < C:
        blocks.append((t, min(NB, C - t)))
        t += NB
    sbs = [[b] for b in blocks]
    if len(sbs) >= 2 and sbs[-1][0][1] < NB:
        tail = sbs.pop()[0]
        sbs[-1].append(tail)
        (t0, w0), (t1, w1) = sbs[-1]
        if w1 < 128:
            w0n = w0 + w1 - 128
            sbs[-1] = [(t0, w0n), (t0 + w0n, 128)]
    return sbs


def build_program(C, h=H, i_dim=I, sim_safe_act=False):
    """Build the SPMD bass program for one expert over C tokens.

    DRAM I/O layouts (all partition-major, pre-packed on host):
      x  [P, KH, C]        fp16   x[p, k, t]  = token t, hidden 128k+p
      wg [MI, P, KH*P]     fp16   wg[m, p, kf] (kf = k*128+f): w_gate.T tiles
      wu [MI, P, KH*P]     fp16   same for w_up
      wd [KH, P, MI*P]     fp16   w_down.T tiles
      y  [P, KH, C]        bf16   y[p, m2, t] = output hidden 128*m2+p
           (bf16 keeps y's ~0.2% quantization noise far under the 2e-2
            gate and halves the final drain + scalar-ring write traffic)
    """
    from concourse import bacc, tile, mybir

    kh = h // P
    mi = i_dim // P
    bf = mybir.dt.float16
    bf16 = mybir.dt.bfloat16
    f32 = mybir.dt.float32
    Silu = mybir.ActivationFunctionType.Silu

    nc = bacc.Bacc(None)
    X = nc.declare_dram_parameter("x", [P, kh, C], bf, isOutput=False)
    WG = nc.declare_dram_parameter("wg", [mi, P, kh * P], bf, isOutput=False)
    WU = nc.declare_dram_parameter("wu", [mi, P, kh * P], bf, isOutput=False)
    WD = nc.declare_dram_parameter("wd", [kh, P, mi * P], bf, isOutput=False)
    Y = nc.declare_dram_parameter("y", [P, kh, C], bf16, isOutput=True)

    with ExitStack() as ctx:
        tc = ctx.enter_context(tile.TileContext(nc))
        xpool = ctx.enter_context(tc.tile_pool(name="xpool", bufs=2))
        wpool = ctx.enter_context(tc.tile_pool(name="wpool", bufs=6))
        dpool = ctx.enter_context(tc.tile_pool(name="dpool", bufs=4))
        hpool = ctx.enter_context(tc.tile_pool(name="hpool", bufs=1))
        apool = ctx.enter_context(tc.tile_pool(name="apool", bufs=3))
        ypool = ctx.enter_context(tc.tile_pool(name="ypool", bufs=3))
        pg_pool = ctx.enter_context(tc.tile_pool(name="pg", bufs=3, space="PSUM"))
        pu_pool = ctx.enter_context(tc.tile_pool(name="pu", bufs=3, space="PSUM"))
        py_pool = ctx.enter_context(tc.tile_pool(name="py", bufs=2, space="PSUM"))

        first_sb = True
        for sb in _superblocks(C):
            # Only sync (qSPDynamicHW) and scalar (qActDynamicHW) are
            # hardware DGE rings; gpsimd DMA is software-DGE and slow.
            q = kh * P // 4
            pre_wg = pre_wu = None
            x_ts = []
            if first_sb:
                # ---- first superblock: interleave the m=0 weight chunks
                # with the x chunks across both HW rings so the first pg
                # chain starts at the ~12us DMA-latency floor instead of
                # queueing all 16 x chunks ahead of the weights (~19us).
                (t0, tn) = sb[0]
                x_t = xpool.tile([P, kh, tn], bf, tag="x_t0", name="x_t0")
                x_ts.append(x_t)
                pre_wg = wpool.tile([P, kh * P], bf, tag="wg_t")
                pre_wu = wpool.tile([P, kh * P], bf, tag="wu_t")
                for j in range(4):
                    nc.sync.dma_start(pre_wg[:, j * q : (j + 1) * q], WG[0, :, j * q : (j + 1) * q])
                    nc.scalar.dma_start(pre_wu[:, j * q : (j + 1) * q], WU[0, :, j * q : (j + 1) * q])
                    for k in range(4 * j, 4 * j + 4):
                        eng = nc.sync if k % 2 == 0 else nc.scalar
                        eng.dma_start(x_t[:, k, :tn], X[:, k, t0 : t0 + tn])
            else:
                # ---- load X for each column group: kh tiles [P, tn]
                for g, (t0, tn) in enumerate(sb):
                    x_t = xpool.tile([P, kh, tn], bf, tag=f"x_t{g}", name=f"x_t{g}")
                    for k in range(kh):
                        eng = nc.scalar if k % 2 == 0 else nc.sync
                        eng.dma_start(x_t[:, k, :tn], X[:, k, t0 : t0 + tn])
                    x_ts.append(x_t)
            first_sb = False

            # ---- mm1/mm2 + silu*mul -> h (one weight pass for all groups)
            h_ts = [
                hpool.tile([P, mi, sb[g][1]], bf, tag=f"h{g}", name=f"h_t{g}")
                for g in range(len(sb))
            ]
            for m in range(mi):
                if m == 0 and pre_wg is not None:
                    wg_t, wu_t = pre_wg, pre_wu
                else:
                    # all weights on sync: it is the one HW-DGE ring with no
                    # compute duties, so its blocking DMA waits hurt nothing
                    wg_t = wpool.tile([P, kh * P], bf, tag="wg_t")
                    for j in range(4):
                        nc.sync.dma_start(wg_t[:, j * q : (j + 1) * q], WG[m, :, j * q : (j + 1) * q])
                    wu_t = wpool.tile([P, kh * P], bf, tag="wu_t")
                    for j in range(4):
                        nc.sync.dma_start(wu_t[:, j * q : (j + 1) * q], WU[m, :, j * q : (j + 1) * q])

                pgs, pus = [], []
                for g, (t0, tn) in enumerate(sb):
                    pg = pg_pool.tile([P, NB], f32, tag="pg")
                    pgs.append(pg)
                    for k in range(kh):
                        nc.tensor.matmul(
                            pg[:, :tn],
                            wg_t[:, k * P : (k + 1) * P],
                            x_ts[g][:, k, :tn],
                            start=(k == 0),
                            stop=(k == kh - 1),
                        )
                for g, (t0, tn) in enumerate(sb):
                    pu = pu_pool.tile([P, NB], f32, tag="pu")
                    pus.append(pu)
                    for k in range(kh):
                        nc.tensor.matmul(
                            pu[:, :tn],
                            wu_t[:, k * P : (k + 1) * P],
                            x_ts[g][:, k, :tn],
                            start=(k == 0),
                            stop=(k == kh - 1),
                        )
                for g, (t0, tn) in enumerate(sb):
                    pg, pu = pgs[g], pus[g]
                    g_act = apool.tile([P, NB], f32, tag="g_act")
                    if sim_safe_act:
                        # silu(g) = g * sigmoid(g); CoreSim lacks the Silu LUT
                        nc.scalar.activation(
                            g_act[:, :tn],
                            pg[:, :tn],
                            mybir.ActivationFunctionType.Sigmoid,
                        )
                        nc.vector.tensor_mul(g_act[:, :tn], g_act[:, :tn], pg[:, :tn])
                    else:
                        nc.scalar.activation(g_act[:, :tn], pg[:, :tn], Silu)
                    nc.vector.tensor_mul(h_ts[g][:, m, :tn], g_act[:, :tn], pu[:, :tn])

            # ---- mm3 -> y (one weight pass for all groups)
            for m2 in range(kh):
                dhalf = mi * P // 2
                wd_t = dpool.tile([P, mi * P], bf, tag="wd_t")
                nc.sync.dma_start(wd_t[:, :dhalf], WD[m2, :, :dhalf])
                nc.sync.dma_start(wd_t[:, dhalf:], WD[m2, :, dhalf:])
                # tail group first so its drain hides behind the main
                # stream — except on the very last m2, where main-first
                # leaves only the small tail tile's drain exposed at the end
                g_order = list(enumerate(sb))
                if m2 < kh - 1:
                    g_order = list(reversed(g_order))
                for g, (t0, tn) in g_order:
                    py = py_pool.tile([P, NB], f32, tag="py")
                    for k2 in range(mi):
                        nc.tensor.matmul(
                            py[:, :tn],
                            wd_t[:, k2 * P : (k2 + 1) * P],
                            h_ts[g][:, k2, :tn],
                            start=(k2 == 0),
                            stop=(k2 == mi - 1),
                        )
                    y_sb = ypool.tile([P, NB], bf16, tag="y_sb")
                    nc.vector.tensor_copy(y_sb[:, :tn], py[:, :tn])
                    nc.scalar.dma_start(Y[:, m2, t0 : t0 + tn], y_sb[:, :tn])

    nc.compile()
    return nc


def _route(xf, gate_w, top_k):
    """Host router: returns per-expert (token_indices, weights)."""
    logits = xf @ gate_w.T.astype(np.float32)  # [T, E]
    m = logits.max(-1, keepdims=True)
    p = np.exp(logits - m)
    p /= p.sum(-1, keepdims=True)
    k = int(top_k)
    if k >= E:
        top_i = np.tile(np.arange(E), (xf.shape[0], 1))
    else:
        top_i = np.argpartition(-p, k, axis=-1)[:, :k]
    top_w = np.take_along_axis(p, top_i, axis=-1)
    top_w = top_w / top_w.sum(-1, keepdims=True)
    idxs, wts = [], []
    for e in range(E):
        sel = top_i == e  # [T, k]
        tok = np.nonzero(sel.any(-1))[0]
        w = (top_w * sel).sum(-1)[tok].astype(np.float32)
        idxs.append(tok)
        wts.append(w)
    return idxs, wts


def _pack_w1(w):  # [I, H] -> [MI, P, KH*P]; lhsT tile (m,k)[p,f] = w[128m+f, 128k+p]
    return np.ascontiguousarray(
        w.reshape(MI, P, KH, P).transpose(0, 3, 2, 1).reshape(MI, P, KH * P)
    )


def _pack_w3(w):  # [H, I] -> [KH, P, MI*P]; lhsT tile (m2,k2)[p,f] = w[128m2+f, 128k2+p]
    return np.ascontiguousarray(
        w.reshape(KH, P, MI, P).transpose(0, 3, 2, 1).reshape(KH, P, MI * P)
    )


def kernel(x, gate_w, w_gate, w_up, w_down, top_k):
    from concourse.bass_utils import run_bass_kernel_spmd

    x = np.asarray(x, dtype=np.float32)
    gate_w = np.asarray(gate_w, dtype=np.float32)
    w_gate = np.asarray(w_gate, dtype=np.float32)
    w_up = np.asarray(w_up, dtype=np.float32)
    w_down = np.asarray(w_down, dtype=np.float32)
    shape = x.shape
    xf = x.reshape(-1, shape[-1])
    T = xf.shape[0]

    idxs, wts = _route(xf, gate_w, top_k)
    C = max(max(len(ix) for ix in idxs), NB)
    C = ((C + 3) // 4) * 4  # pad only to 4 (8B DMA lines) — C is the roofline

    nc = build_program(C)

    xf_bf = xf.astype(DT)
    in_maps = []
    for e in range(E):
        tok = idxs[e]
        xg = np.zeros((C, H), dtype=DT)
        xg[: len(tok)] = xf_bf[tok]
        # [C, H] -> x[p, k, t] = xg[t, 128k+p]
        xp = np.ascontiguousarray(xg.reshape(C, KH, P).transpose(2, 1, 0))
        in_maps.append(
            {
                "x": xp,
                "wg": _pack_w1(w_gate[e].astype(DT)),
                "wu": _pack_w1(w_up[e].astype(DT)),
                "wd": _pack_w3(w_down[e].astype(DT)),
            }
        )

    trace = bool(os.environ.get("BASS_TRACE"))
    if trace:
        try:
            import antenv.axon_hooks  # noqa: F401  (trace path needs it under axon)
        except ImportError:
            trace = False
            os.environ["BASS_NEVER_TRACE"] = "1"
    res = run_bass_kernel_spmd(nc, in_maps, list(range(E)), trace=trace)
    globals()["LAST_RESULT"] = res

    out = np.zeros((T, H), dtype=np.float32)
    for e in range(E):
        tok = idxs[e]
        y = res.results[e]["y"].astype(np.float32)  # [P, KH, C] bf16 on device
        yt = y.transpose(2, 1, 0).reshape(C, H)[: len(tok)]
        out[tok] += yt * wts[e][:, None]
    return out.reshape(shape)



# revision 36
# speedup vs baseline: 1.0022x; 1.0007x over previous
"""MoE layer (8 experts, top-2) on 8 Trainium2 NeuronCores, expert-parallel.

Strategy
--------
Host (dispatch): compute router logits/top-k on host, gather each expert's
tokens into a padded capacity buffer (C = max expert load, 4-aligned),
pre-pack activations/weights into the exact SBUF tile layout
(partition-major) in fp16.
Device (one expert per core, SPMD): Y_e = w_down[e] @ (silu(w_gate[e] @ x_e)
* (w_up[e] @ x_e)) over the expert's C gathered tokens; all matmuls fp16
inputs with fp32 PSUM accumulation (fp16 runs at full PE rate like bf16 but
with 8x finer mantissa). Token columns are processed in 512-wide blocks;
the remainder is merged into the last block's weight pass and the merged
pair rebalanced so no block drops below the ~69-col instruction-issue
floor of the PE (measured: the tensor engine issues matmuls no faster
than ~29ns apart, and a 512-col fp16 matmul takes 216ns = 2.37GHz).
Host (combine): scatter-add per-token routing-weighted outputs.

Trace-derived tuning notes (this exact workload, TRN2):
- Only sync (qSPDynamicHW) and scalar (qActDynamicHW) issue HW-DGE DMAs;
  gpsimd DMA is software-DGE and far too slow for streaming.
- All heavy weight streams must ride sync: scalar runs the ACTIVATE
  (silu) instructions, and DMA issues blocked on semaphore-slot reuse
  would delay them, stalling PSUM recycling and the PE.
- Every weight pass needs per-m-iter compute >= the sync ring's ~5-6us
  per-m weight delivery, hence all non-tail blocks stay 512 wide.
"""

import os
import numpy as np
from contextlib import ExitStack

H = 2048
I = 5632
E = 8
P = 128
NB = 512  # token block (matmul free dim / PSUM bank)

KH = H // P   # 16  k-tiles over H
MI = I // P   # 44  m-tiles over I

DT = np.float16  # fp16: PE full rate like bf16, 8x finer mantissa


def _superblocks(C):
    """Column groups; a trailing remainder (<NB) is merged into the last
    full block so both share one pass over the weights.

    Matmuls below ~69 cols are bound by the 29ns instruction-issue floor
    (29ns buys 69 cols at 2.37GHz), so a skinny tail wastes PE time.
    Non-tail passes must stay 512 wide: a narrower pass consumes weights
    faster than the single sync HW-DGE ring delivers (~5-6us per m-iter),
    and the weight stream cannot ride the scalar ring without burying the
    ACTIVATE instructions behind blocking DMA waits.  So rebalance only
    inside the merged tail pass: [512, t<128] -> [384+t, 128]."""
    blocks = []
    t = 0
    while t < C:
        blocks.append((t, min(NB, C - t)))
        t += NB
    sbs = [[b] for b in blocks]
    if len(sbs) >= 2 and sbs[-1][0][1] < NB:
        tail = sbs.pop()[0]
        sbs[-1].append(tail)
        (t0, w0), (t1, w1) = sbs[-1]
        if w1 < 128:
            w0n = w0 + w1 - 128
            sbs[-1] = [(t0, w0n), (t0 + w0n, 128)]
    return sbs


def build_program(C, h=H, i_dim=I, sim_safe_act=False):
    """Build the SPMD bass program for one expert over C tokens.

    DRAM I/O layouts (all partition-major, pre-packed on host):
      x  [P, KH, C]        fp16   x[p, k, t]  = token t, hidden 128k+p
      wg [MI, P, KH*P]     fp16   wg[m, p, kf] (kf = k*128+f): w_gate.T tiles
      wu [MI, P, KH*P]     fp16   same for w_up
      wd [KH, P, MI*P]     fp16   w_down.T tiles
      y  [P, KH, C]        bf16   y[p, m2, t] = output hidden 128*m2+p
           (bf16 keeps y's ~0.2% quantization noise far under the 2e-2
            gate and halves the final drain + scalar-ring write traffic)
    """
    from concourse import bacc, tile, mybir

    kh = h // P
    mi = i_dim // P
    bf = mybir.dt.float16
    bf16 = mybir.dt.bfloat16
    f32 = mybir.dt.float32
    Silu = mybir.ActivationFunctionType.Silu

    nc = bacc.Bacc(None)
    X = nc.declare_dram_parameter("x", [P, kh, C], bf, isOutput=False)
    WG = nc.declare_dram_parameter("wg", [mi, P, kh * P], bf, isOutput=False)
    WU = nc.declare_dram_parameter("wu", [mi, P, kh * P], bf, isOutput=False)
    WD = nc.declare_dram_parameter("wd", [kh, P, mi * P], bf, isOutput=False)
    Y = nc.declare_dram_parameter("y", [P, kh, C], bf16, isOutput=True)

    with ExitStack() as ctx:
        tc = ctx.enter_context(tile.TileContext(nc))
        xpool = ctx.enter_context(tc.tile_pool(name="xpool", bufs=2))
        wpool = ctx.enter_context(tc.tile_pool(name="wpool", bufs=6))
        dpool = ctx.enter_context(tc.tile_pool(name="dpool", bufs=4))
        hpool = ctx.enter_context(tc.tile_pool(name="hpool", bufs=1))
        apool = ctx.enter_context(tc.tile_pool(name="apool", bufs=3))
        ypool = ctx.enter_context(tc.tile_pool(name="ypool", bufs=3))
        pg_pool = ctx.enter_context(tc.tile_pool(name="pg", bufs=3, space="PSUM"))
        pu_pool = ctx.enter_context(tc.tile_pool(name="pu", bufs=3, space="PSUM"))
        py_pool = ctx.enter_context(tc.tile_pool(name="py", bufs=2, space="PSUM"))

        first_sb = True
        for sb in _superblocks(C):
            # Only sync (qSPDynamicHW) and scalar (qActDynamicHW) are
            # hardware DGE rings; gpsimd DMA is software-DGE and slow.
            q = kh * P // 4
            pre_wg = pre_wu = None
            x_ts = []
            if first_sb:
                # ---- first superblock: interleave the m=0 weight chunks
                # with the x chunks across both HW rings so the first pg
                # chain starts at the ~12us DMA-latency floor instead of
                # queueing all 16 x chunks ahead of the weights (~19us).
                (t0, tn) = sb[0]
                x_t = xpool.tile([P, kh, tn], bf, tag="x_t0", name="x_t0")
                x_ts.append(x_t)
                pre_wg = wpool.tile([P, kh * P], bf, tag="wg_t")
                pre_wu = wpool.tile([P, kh * P], bf, tag="wu_t")
                for j in range(4):
                    nc.sync.dma_start(pre_wg[:, j * q : (j + 1) * q], WG[0, :, j * q : (j + 1) * q])
                    nc.scalar.dma_start(pre_wu[:, j * q : (j + 1) * q], WU[0, :, j * q : (j + 1) * q])
                    for k in range(4 * j, 4 * j + 4):
                        eng = nc.sync if k % 2 == 0 else nc.scalar
                        eng.dma_start(x_t[:, k, :tn], X[:, k, t0 : t0 + tn])
            else:
                # ---- load X for each column group: kh tiles [P, tn]
                for g, (t0, tn) in enumerate(sb):
                    x_t = xpool.tile([P, kh, tn], bf, tag=f"x_t{g}", name=f"x_t{g}")
                    for k in range(kh):
                        eng = nc.scalar if k % 2 == 0 else nc.sync
                        eng.dma_start(x_t[:, k, :tn], X[:, k, t0 : t0 + tn])
                    x_ts.append(x_t)
            first_sb = False

            # ---- mm1/mm2 + silu*mul -> h (one weight pass for all groups)
            h_ts = [
                hpool.tile([P, mi, sb[g][1]], bf, tag=f"h{g}", name=f"h_t{g}")
                for g in range(len(sb))
            ]
            for m in range(mi):
                if m == 0 and pre_wg is not None:
                    wg_t, wu_t = pre_wg, pre_wu
                else:
                    # all weights on sync: it is the one HW-DGE ring with no
                    # compute duties, so its blocking DMA waits hurt nothing
                    wg_t = wpool.tile([P, kh * P], bf, tag="wg_t")
                    for j in range(4):
                        nc.sync.dma_start(wg_t[:, j * q : (j + 1) * q], WG[m, :, j * q : (j + 1) * q])
                    wu_t = wpool.tile([P, kh * P], bf, tag="wu_t")
                    for j in range(4):
                        nc.sync.dma_start(wu_t[:, j * q : (j + 1) * q], WU[m, :, j * q : (j + 1) * q])

                pgs, pus = [], []
                for g, (t0, tn) in enumerate(sb):
                    pg = pg_pool.tile([P, NB], f32, tag="pg")
                    pgs.append(pg)
                    for k in range(kh):
                        nc.tensor.matmul(
                            pg[:, :tn],
                            wg_t[:, k * P : (k + 1) * P],
                            x_ts[g][:, k, :tn],
                            start=(k == 0),
                            stop=(k == kh - 1),
                        )
                for g, (t0, tn) in enumerate(sb):
                    pu = pu_pool.tile([P, NB], f32, tag="pu")
                    pus.append(pu)
                    for k in range(kh):
                        nc.tensor.matmul(
                            pu[:, :tn],
                            wu_t[:, k * P : (k + 1) * P],
                            x_ts[g][:, k, :tn],
                            start=(k == 0),
                            stop=(k == kh - 1),
                        )
                for g, (t0, tn) in enumerate(sb):
                    pg, pu = pgs[g], pus[g]
                    g_act = apool.tile([P, NB], f32, tag="g_act")
                    if sim_safe_act:
                        # silu(g) = g * sigmoid(g); CoreSim lacks the Silu LUT
                        nc.scalar.activation(
                            g_act[:, :tn],
                            pg[:, :tn],
                            mybir.ActivationFunctionType.Sigmoid,
                        )
                        nc.vector.tensor_mul(g_act[:, :tn], g_act[:, :tn], pg[:, :tn])
                    else:
                        nc.scalar.activation(g_act[:, :tn], pg[:, :tn], Silu)
                    nc.vector.tensor_mul(h_ts[g][:, m, :tn], g_act[:, :tn], pu[:, :tn])

            # ---- mm3 -> y (one weight pass for all groups)
            for m2 in range(kh):
                dhalf = mi * P // 2
                wd_t = dpool.tile([P, mi * P], bf, tag="wd_t")
                nc.sync.dma_start(wd_t[:, :dhalf], WD[m2, :, :dhalf])
                nc.sync.dma_start(wd_t[:, dhalf:], WD[m2, :, dhalf:])
                # tail group first so its drain hides behind the main
                # stream — except on the very last m2, where main-first
                # leaves only the small tail tile's drain exposed at the end
                g_order = list(enumerate(sb))
                if m2 < kh - 1:
                    g_order = list(reversed(g_order))
                for g, (t0, tn) in g_order:
                    py = py_pool.tile([P, NB], f32, tag="py")
                    for k2 in range(mi):
                        nc.tensor.matmul(
                            py[:, :tn],
                            wd_t[:, k2 * P : (k2 + 1) * P],
                            h_ts[g][:, k2, :tn],
                            start=(k2 == 0),
                            stop=(k2 == mi - 1),
                        )
                    y_sb = ypool.tile([P, NB], bf16, tag="y_sb")
                    nc.vector.tensor_copy(y_sb[:, :tn], py[:, :tn])
                    nc.scalar.dma_start(Y[:, m2, t0 : t0 + tn], y_sb[:, :tn])

    nc.compile()
    return nc


def _route(xf, gate_w, top_k):
    """Host router: returns per-expert (token_indices, weights)."""
    logits = xf @ gate_w.T.astype(np.float32)  # [T, E]
    m = logits.max(-1, keepdims=True)
    p = np.exp(logits - m)
    p /= p.sum(-1, keepdims=True)
    k = int(top_k)
    if k >= E:
        top_i = np.tile(np.arange(E), (xf.shape[0], 1))
    else:
        top_i = np.argpartition(-p, k, axis=-1)[:, :k]
    top_w = np.take_along_axis(p, top_i, axis=-1)
    top_w = top_w / top_w.sum(-1, keepdims=True)
    idxs, wts = [], []
    for e in range(E):
        sel = top_i == e  # [T, k]
        tok = np.nonzero(sel.any(-1))[0]
        w = (top_w * sel).sum(-1)[tok].astype(np.float32)
        idxs.append(tok)
        wts.append(w)
    return idxs, wts


def _pack_w1(w):  # [I, H] -> [MI, P, KH*P]; lhsT tile (m,k)[p,f] = w[128m+f, 128k+p]
    return np.ascontiguousarray(
        w.reshape(MI, P, KH, P).transpose(0, 3, 2, 1).reshape(MI, P, KH * P)
    )


def _pack_w3(w):  # [H, I] -> [KH, P, MI*P]; lhsT tile (m2,k2)[p,f] = w[128m2+f, 128k2+p]
    return np.ascontiguousarray(
        w.reshape(KH, P, MI, P).transpose(0, 3, 2, 1).reshape(KH, P, MI * P)
    )


def kernel(x, gate_w, w_gate, w_up, w_down, top_k):
    from concourse.bass_utils import run_bass_kernel_spmd

    x = np.asarray(x, dtype=np.float32)
    gate_w = np.asarray(gate_w, dtype=np.float32)
    w_gate = np.asarray(w_gate, dtype=np.float32)
    w_up = np.asarray(w_up, dtype=np.float32)
    w_down = np.asarray(w_down, dtype=np.float32)
    shape = x.shape
    xf = x.reshape(-1, shape[-1])
    T = xf.shape[0]

    idxs, wts = _route(xf, gate_w, top_k)
    C = max(max(len(ix) for ix in idxs), NB)
    C = ((C + 3) // 4) * 4  # pad only to 4 (8B DMA lines) — C is the roofline

    nc = build_program(C)

    xf_bf = xf.astype(DT)
    in_maps = []
    for e in range(E):
        tok = idxs[e]
        xg = np.zeros((C, H), dtype=DT)
        xg[: len(tok)] = xf_bf[tok]
        # [C, H] -> x[p, k, t] = xg[t, 128k+p]
        xp = np.ascontiguousarray(xg.reshape(C, KH, P).transpose(2, 1, 0))
        in_maps.append(
            {
                "x": xp,
                "wg": _pack_w1(w_gate[e].astype(DT)),
                "wu": _pack_w1(w_up[e].astype(DT)),
                "wd": _pack_w3(w_down[e].astype(DT)),
            }
        )

    trace = bool(os.environ.get("BASS_TRACE"))
    if trace:
        try:
            import antenv.axon_hooks  # noqa: F401  (trace path needs it under axon)
        except ImportError:
            trace = False
            os.environ["BASS_NEVER_TRACE"] = "1"
    res = run_bass_kernel_spmd(nc, in_maps, list(range(E)), trace=trace)
    globals()["LAST_RESULT"] = res

    out = np.zeros((T, H), dtype=np.float32)
    for e in range(E):
        tok = idxs[e]
        y = res.results[e]["y"].astype(np.float32)  # [P, KH, C] bf16 on device
        yt = y.transpose(2, 1, 0).reshape(C, H)[: len(tok)]
        out[tok] += yt * wts[e][:, None]
    return out.reshape(shape)

